# revision 1
# baseline (speedup 1.0000x reference)
"""Trainium2 Bass kernel for LocalSquaredDistanceLayer (shapelet min-distance).

Math (matching the reference exactly):
  x_norm   = z-normalize x over time per (batch, channel)
  kern     = z-normalize kernel per shapelet over (KSZ, C)
  For output element out[b, t, k'] with k' = 4*ch + j (ch = k'//4, j = k'%4):
     w = x_norm[b, t+8j : t+8j+8, ch]               (8 consecutive samples)
     out[b,t,k'] = min_s || w - kern[s, k', :] ||^2
  (This is the tf.extract_patches transpose/reshape identity: the patch
   vector for k' is 8 consecutive time samples of channel k'//4 offset 8*(k'%4).)

Device algorithm per core (2 batches per core, kernel replicated):
  - transpose x to per-(b,ch) time signals, z-normalize, square
  - build Hankel tiles H[sig] (65, 512): rows 0-31 x-shifts, 32-63 x^2 shifts,
    row 64 ones
  - build filter tiles F[ch] (65, 256): block-diagonal taps (-2*kern^T),
    ones blocks (for the x^2 window sum), K2 row (||kern||^2) so that one
    matmul psum[t, 4ch+j*? ...] = full squared distance
  - matmul per (b, tchunk, ch): psum (128, 8*256) = distances for 32 k'
    groups x 64 shapelets
  - min-reduce over the innermost 64 (shapelets), DMA out
"""

import sys

for _p in ("/opt/trn_rl_repo",):
    if _p not in sys.path:
        sys.path.insert(0, _p)

import numpy as np

B, T, C = 16, 512, 8
S, KSZ = 64, 32
TOUT = T - KSZ + 1  # 481
NCORES = 8
BPC = B // NCORES  # batches per core
NSIG = BPC * C  # signals per core
EPS = 1e-8
XPAD = 544  # padded signal length (hankel reads up to 511+31)

_cache = {}


def _rap(base, dims):
    """Raw AP at base slice's offset with explicit [step, count] dims (flat elems)."""
    import concourse.bass as bass

    return bass.AP(tensor=base.tensor, offset=base.offset, ap=[list(d) for d in dims])


def _build_nc():
    import concourse.bass as bass
    import concourse.bacc as bacc
    import concourse.tile as tile
    from concourse import mybir
    from concourse.masks import make_identity
    from contextlib import ExitStack

    f32 = mybir.dt.float32
    AX = mybir.AxisListType
    OP = mybir.AluOpType
    ACT = mybir.ActivationFunctionType

    nc = bacc.Bacc("TRN2", target_bir_lowering=False, debug=False)
    x_d = nc.dram_tensor("x", [BPC, T, C], f32, kind="ExternalInput").ap()
    k_d = nc.dram_tensor("kernel", [S, KSZ, C], f32, kind="ExternalInput").ap()
    o_d = nc.dram_tensor("out", [BPC, TOUT, KSZ], f32, kind="ExternalOutput").ap()

    with tile.TileContext(nc) as tc, ExitStack() as ctx:
        const = ctx.enter_context(tc.tile_pool(name="const", bufs=1))
        outp = ctx.enter_context(tc.tile_pool(name="outp", bufs=4))
        dram = ctx.enter_context(tc.tile_pool(name="dram", bufs=1, space="DRAM"))

        ident = const.tile([128, 128], f32, tag="ident")
        make_identity(nc, ident[:])

        F_tiles = [const.tile([65, 256], f32, tag=f"F{ch}", name=f"F{ch}") for ch in range(C)]
        ones8 = const.tile([8, 64], f32, tag="ones8")
        nc.vector.memset(ones8[:], 1.0)
        onesD = dram.tile([8, 64], f32, tag="onesD")
        nc.sync.dma_start(out=onesD[:], in_=ones8[:])
        H_tiles = [const.tile([65, 512], f32, tag=f"H{s}", name=f"H{s}") for s in range(NSIG)]
        Xn = const.tile([NSIG, XPAD], f32, tag="Xn")
        X2n = const.tile([NSIG, XPAD], f32, tag="X2n")

        with tc.tile_pool(name="pprep", bufs=1, space="PSUM") as pprep, \
             tc.tile_pool(name="ldp", bufs=2) as ldp:
            # ---- kernel prep ----
            KN = const.tile([S, KSZ * C], f32, tag="KN")
            nc.sync.dma_start(out=KN[:], in_=k_d.rearrange("s k c -> s (k c)"))
            kst = ldp.tile([S, nc.vector.BN_STATS_DIM], f32, tag="kst")
            nc.vector.bn_stats(out=kst[:], in_=KN[:])
            mvk = ldp.tile([S, nc.vector.BN_AGGR_DIM], f32, tag="mvk")
            nc.vector.bn_aggr(out=mvk[:], in_=kst[:])
            kstd = ldp.tile([S, 1], f32, tag="kstd")
            nc.scalar.activation(out=kstd[:], in_=mvk[:, 1:2], func=ACT.Sqrt)
            nc.vector.tensor_scalar_add(kstd[:], kstd[:], EPS)
            krstd = ldp.tile([S, 1], f32, tag="krstd")
            nc.vector.reciprocal(out=krstd[:], in_=kstd[:])
            # KNm = -2 * (KN - mean) * rstd  => scale=-2*rstd, bias=2*mean*rstd
            kscale = ldp.tile([S, 1], f32, tag="kscale")
            nc.vector.tensor_scalar_mul(kscale[:], krstd[:], -2.0)
            kbias = ldp.tile([S, 1], f32, tag="kbias")
            nc.vector.scalar_tensor_tensor(
                out=kbias[:], in0=mvk[:, 0:1], scalar=2.0, in1=krstd[:],
                op0=OP.mult, op1=OP.mult)
            KNm = const.tile([S, KSZ * C], f32, tag="KNm")
            nc.vector.tensor_scalar(
                out=KNm[:], in0=KN[:], scalar1=kscale[:], scalar2=kbias[:],
                op0=OP.mult, op1=OP.add)
            # K2[s,k'] = sum_c kern_n^2 = 0.25 * sum_c KNm^2
            KN2 = ldp.tile([S, KSZ * C], f32, tag="KN2")
            nc.scalar.activation(out=KN2[:], in_=KNm[:], func=ACT.Square)
            K2sn = const.tile([S, KSZ], f32, tag="K2sn")
            nc.vector.tensor_reduce(
                out=K2sn[:], in_=KN2[:].rearrange("s (k c) -> s k c", c=C),
                axis=AX.X, op=OP.add)
            nc.vector.tensor_scalar_mul(K2sn[:], K2sn[:], 0.25)

            # transpose KNm (64, 256) -> TP (8, 32*64) psum, slice per k'
            TP = pprep.tile([8, KSZ * S], f32, tag="TP")
            for kp in range(KSZ):
                nc.tensor.transpose(
                    TP[:, kp * S:(kp + 1) * S], KNm[:, kp * C:(kp + 1) * C],
                    ident[0:S, 0:S])
            K2T = pprep.tile([KSZ, S], f32, tag="K2T")
            nc.tensor.transpose(K2T[:], K2sn[:], ident[0:S, 0:S])
            # psum is not DMA-addressable: stage to SBUF via ACT copies
            Fx = ldp.tile([8, KSZ * S], f32, tag="Fx")
            nc.scalar.copy(out=Fx[:], in_=TP[:])
            K2sb = ldp.tile([KSZ, S], f32, tag="K2sb")
            nc.scalar.copy(out=K2sb[:], in_=K2T[:])
            FxD = dram.tile([8, KSZ * S], f32, tag="FxD")
            nc.sync.dma_start(out=FxD[:], in_=Fx[:])
            K2D = dram.tile([KSZ, S], f32, tag="K2D")
            nc.sync.dma_start(out=K2D[:], in_=K2sb[:])

            # ---- F tile zero-fill (scatter happens after the barrier) ----
            for ch in range(C):
                nc.vector.memset(F_tiles[ch][:], 0.0)

            # ---- x load + transpose to signals ----
            PX = pprep.tile([8, BPC * T], f32, tag="PX")
            for b in range(BPC):
                for cc in range(4):
                    X0 = ldp.tile([128, C], f32, tag="X0")
                    nc.sync.dma_start(out=X0[:], in_=x_d[b, cc * 128:(cc + 1) * 128, :])
                    nc.tensor.transpose(
                        PX[:, b * T + cc * 128: b * T + (cc + 1) * 128], X0[:],
                        ident[:, :])
            Xst = ldp.tile([8, BPC * T], f32, tag="Xst")
            nc.scalar.copy(out=Xst[:], in_=PX[:])
            Xsig = ldp.tile([NSIG, T], f32, tag="Xsig")
            for b in range(BPC):
                nc.sync.dma_start(
                    out=Xsig[b * C:(b + 1) * C, :], in_=Xst[:, b * T:(b + 1) * T])

            # ---- x normalize ----
            xst = ldp.tile([NSIG, nc.vector.BN_STATS_DIM], f32, tag="xst")
            nc.vector.bn_stats(out=xst[:], in_=Xsig[:])
            mvx = ldp.tile([NSIG, nc.vector.BN_AGGR_DIM], f32, tag="mvx")
            nc.vector.bn_aggr(out=mvx[:], in_=xst[:])
            xstd = ldp.tile([NSIG, 1], f32, tag="xstd")
            nc.scalar.activation(out=xstd[:], in_=mvx[:, 1:2], func=ACT.Sqrt)
            nc.vector.tensor_scalar_add(xstd[:], xstd[:], EPS)
            xrstd = ldp.tile([NSIG, 1], f32, tag="xrstd")
            nc.vector.reciprocal(out=xrstd[:], in_=xstd[:])
            xbias = ldp.tile([NSIG, 1], f32, tag="xbias")
            nc.vector.scalar_tensor_tensor(
                out=xbias[:], in0=mvx[:, 0:1], scalar=-1.0, in1=xrstd[:],
                op0=OP.mult, op1=OP.mult)
            nc.vector.memset(Xn[:], 0.0)
            nc.vector.memset(X2n[:], 0.0)
            nc.vector.tensor_scalar(
                out=Xn[:, 0:T], in0=Xsig[:], scalar1=xrstd[:], scalar2=xbias[:],
                op0=OP.mult, op1=OP.add)
            nc.scalar.activation(out=X2n[:, 0:T], in_=Xn[:, 0:T], func=ACT.Square)

            # ---- stage normalized signals to DRAM; H ones rows ----
            XnD = dram.tile([NSIG, XPAD], f32, tag="XnD")
            nc.sync.dma_start(out=XnD[:], in_=Xn[:])
            X2nD = dram.tile([NSIG, XPAD], f32, tag="X2nD")
            nc.sync.dma_start(out=X2nD[:], in_=X2n[:])
            for sig in range(NSIG):
                nc.vector.memset(H_tiles[sig][64:65, :], 1.0)

            # ---- single sync point: all staging/memsets above, all
            # scatter DMAs below (keeps per-DMA wait counts at 1) ----
            tc.strict_bb_all_engine_barrier()

            # ---- F tile scatter ----
            for ch in range(C):
                Fc = F_tiles[ch]
                for j in range(4):
                    kp = 4 * ch + j
                    # tap block: F[8j+c, 64j+s] = -2*kern_n[s, 4ch+j, c]
                    nc.sync.dma_start(
                        out=Fc[8 * j:8 * j + 8, S * j:S * (j + 1)],
                        in_=FxD[:, kp * S:(kp + 1) * S])
                    # ones block for x^2 rows
                    nc.sync.dma_start(
                        out=Fc[32 + 8 * j:40 + 8 * j, S * j:S * (j + 1)],
                        in_=onesD[:])
                    # K2 row segment
                    nc.sync.dma_start(
                        out=Fc[64:65, S * j:S * (j + 1)],
                        in_=K2D[kp:kp + 1, :])

            # ---- H tiles (hankels via DRAM shifted reads) ----
            for sig in range(NSIG):
                Hs = H_tiles[sig]
                nc.sync.dma_start(
                    out=Hs[0:KSZ, :],
                    in_=_rap(XnD[sig:sig + 1, 0:1], [[1, KSZ], [1, T]]))
                nc.sync.dma_start(
                    out=Hs[KSZ:2 * KSZ, :],
                    in_=_rap(X2nD[sig:sig + 1, 0:1], [[1, KSZ], [1, T]]))

            # funnel all scatter-DMA completions through one sync point so
            # the matmuls each carry a single wait
            tc.strict_bb_all_engine_barrier()

        # ---- main: matmuls + min-reduce + store ----
        with tc.tile_pool(name="pmm", bufs=2, space="PSUM") as pmm:
            for b in range(BPC):
                for cc in range(4):
                    c0 = cc * 128
                    cnt = 128 if cc < 3 else TOUT - 3 * 128
                    acc = pmm.tile([128, C * 256], f32, tag="acc")
                    for ch in range(C):
                        nc.tensor.matmul(
                            acc[:, ch * 256:(ch + 1) * 256],
                            lhsT=H_tiles[b * C + ch][:, c0:c0 + 128],
                            rhs=F_tiles[ch][:],
                            start=True, stop=True)
                    PM = outp.tile([128, KSZ], f32, tag="PM")
                    nc.vector.tensor_reduce(
                        out=PM[:],
                        in_=acc[:].rearrange("p (g s) -> p g s", s=S),
                        axis=AX.X, op=OP.min)
                    nc.sync.dma_start(
                        out=o_d[b, c0:c0 + cnt, :], in_=PM[0:cnt, :])

    nc.compile()
    return nc


def get_nc():
    if "nc" not in _cache:
        _cache["nc"] = _build_nc()
    return _cache["nc"]


def kernel(x: np.ndarray, kernel: np.ndarray) -> np.ndarray:
    from concourse.bass_utils import run_bass_kernel_spmd

    nc = get_nc()
    x = np.ascontiguousarray(x, dtype=np.float32)
    kern = np.ascontiguousarray(kernel, dtype=np.float32)
    in_maps = [
        {"x": x[i * BPC:(i + 1) * BPC], "kernel": kern} for i in range(NCORES)
    ]
    res = run_bass_kernel_spmd(nc, in_maps, core_ids=list(range(NCORES)))
    return np.concatenate([r["out"] for r in res.results], axis=0)


if __name__ == "__main__":
    rng = np.random.default_rng(0)
    x = rng.standard_normal((B, T, C), dtype=np.float32)
    k = rng.uniform(-0.05, 0.05, (S, KSZ, C)).astype(np.float32)
    out = kernel(x=x, kernel=k)
    print(out.shape, out.dtype)



# revision 12
# speedup vs baseline: 2.5777x; 2.5777x over previous
"""Trainium2 Bass kernel for LocalSquaredDistanceLayer (shapelet min-distance).

Math (matches reference):
  out[b,t,k'] = min_s || xn[b, t+8j : t+8j+8, ch] - kern_n[s, k', :] ||^2
  with k' = 4*ch + j, xn z-normalized per (b,ch) over time, kern_n
  z-normalized per shapelet over (KSZ, C).

Precision scheme: the distance is expanded as sum x^2 - 2xk + k^2 inside one
fp16 matmul, but all three terms derive from the SAME fp16-quantized x-hat /
k-hat so psum is exactly ||x-hat - k-hat||^2 (fp16 products are exact in fp32
psum).  The large-magnitude carriers (x^2 window terms, ||k||^2) are encoded
as fp16 hi+lo pairs so no cancellation-amplified rounding enters.  Error is
~2*sqrt(D)*||quantization|| ~ 1e-3, safely under the 2e-2 gate even at the
smallest observed minima (~0.11).

Per-core layout (2 batches/core, kernel replicated; 8 cores data-parallel):
  Hbig [98, 16*512] fp16, one 512-col block per signal (b,ch):
    rows 0-31  xn-hat shifted 0..31      rows 32-63 x2_hi shifted
    rows 64-95 x2_lo shifted             rows 96-97 ones (K2 hi/lo carriers)
  F_all [98, 2048] fp16, col(ch,j,s) = ch*256 + j*64 + s:
    rows 8j+c: -2*k-hat taps   rows 32+8j+c / 64+8j+c: window-power taps (1.0)
    rows 96/97: K2 hi/lo
  Per (b, t-chunk): 8 matmuls (one per ch) -> psum [128, 2048], DVE min-reduce
  over the innermost 64 shapelets, DMA out.
"""

import sys

for _p in ("/opt/trn_rl_repo",):
    if _p not in sys.path:
        sys.path.insert(0, _p)

import numpy as np

B, T, C = 16, 512, 8
S, KSZ = 64, 32
TOUT = T - KSZ + 1  # 481
NCORES = 8
BPC = B // NCORES  # batches per core
NSIG = BPC * C  # signals per core
EPS = 1e-8
SIGW = 1728  # staged fp16 signal row: [xn(576) | x2_hi(576) | x2_lo(576)]

_cache = {}


def _rap(base, dims):
    """Raw AP at base slice's offset with explicit [step, count] dims (elems)."""
    import concourse.bass as bass

    return bass.AP(tensor=base.tensor, offset=base.offset, ap=[list(d) for d in dims])


def _build_nc():
    import concourse.bass as bass
    import concourse.bacc as bacc
    import concourse.tile as tile
    from concourse import mybir
    from concourse.masks import make_identity
    from contextlib import ExitStack

    f32 = mybir.dt.float32
    f16 = mybir.dt.float16
    AX = mybir.AxisListType
    OP = mybir.AluOpType
    ACT = mybir.ActivationFunctionType

    nc = bacc.Bacc("TRN2", target_bir_lowering=False, debug=False)
    x_d = nc.dram_tensor("x", [BPC, T, C], f32, kind="ExternalInput").ap()
    k_d = nc.dram_tensor("kernel", [S, KSZ, C], f32, kind="ExternalInput").ap()
    o_d = nc.dram_tensor("out", [BPC, TOUT, KSZ], f32, kind="ExternalOutput").ap()

    with tile.TileContext(nc) as tc, ExitStack() as ctx:
        const = ctx.enter_context(tc.tile_pool(name="const", bufs=1))
        outp = ctx.enter_context(tc.tile_pool(name="outp", bufs=3))
        dram = ctx.enter_context(tc.tile_pool(name="dram", bufs=1, space="DRAM"))

        ident = const.tile([128, 128], f32, tag="ident")
        make_identity(nc, ident[:])
        identh = const.tile([S, S], f16, tag="identh")
        make_identity(nc, identh[:])

        F_all = const.tile([98, 2048], f16, tag="F_all")
        Hbig = const.tile([98, NSIG * T], f16, tag="Hbig")
        Xnb = const.tile([NSIG, SIGW], f16, tag="Xnb")

        XnD = dram.tile([NSIG, SIGW], f16, tag="XnD")
        OnD = dram.tile([NSIG, 1024], f16, tag="OnD")
        FxD = dram.tile([128, 128], f16, tag="FxD")
        K2D = dram.tile([2, 2048], f16, tag="K2D")

        with tc.tile_pool(name="pprep", bufs=1, space="PSUM") as pprep, \
             tc.tile_pool(name="ldp", bufs=2) as ldp:
            # ---------- shapelet-kernel chain ----------
            KN = ldp.tile([S, KSZ * C], f32, tag="KN")
            nc.sync.dma_start(out=KN[:], in_=k_d.rearrange("s k c -> s (k c)"))
            kst = ldp.tile([S, nc.vector.BN_STATS_DIM], f32, tag="kst")
            nc.vector.bn_stats(out=kst[:], in_=KN[:])
            mvk = ldp.tile([S, nc.vector.BN_AGGR_DIM], f32, tag="mvk")
            nc.vector.bn_aggr(out=mvk[:], in_=kst[:])
            kstd = ldp.tile([S, 1], f32, tag="kstd")
            nc.scalar.activation(out=kstd[:], in_=mvk[:, 1:2], func=ACT.Sqrt)
            nc.vector.tensor_scalar_add(kstd[:], kstd[:], EPS)
            krstd = ldp.tile([S, 1], f32, tag="krstd")
            nc.vector.reciprocal(out=krstd[:], in_=kstd[:])
            # KNm = -2 * (KN - mean) * rstd, then quantize to fp16 (k-hat taps)
            kscale = ldp.tile([S, 1], f32, tag="kscale")
            nc.vector.tensor_scalar_mul(kscale[:], krstd[:], -2.0)
            kbias = ldp.tile([S, 1], f32, tag="kbias")
            nc.vector.scalar_tensor_tensor(
                out=kbias[:], in0=mvk[:, 0:1], scalar=2.0, in1=krstd[:],
                op0=OP.mult, op1=OP.mult)
            KNmh = ldp.tile([S, KSZ * C], f16, tag="KNmh")
            nc.vector.tensor_scalar(
                out=KNmh[:], in0=KN[:], scalar1=kscale[:], scalar2=kbias[:],
                op0=OP.mult, op1=OP.add)
            # K2[s,k'] = 0.25 * sum_c KNmh^2 in fp32 (exact squares of taps)
            KN2 = ldp.tile([S, KSZ * C], f32, tag="KN2")
            nc.scalar.activation(out=KN2[:], in_=KNmh[:], func=ACT.Square)
            K2sn = ldp.tile([S, KSZ], f32, tag="K2sn")
            nc.vector.tensor_reduce(
                out=K2sn[:], in_=KN2[:].rearrange("s (k c) -> s k c", c=C),
                axis=AX.X, op=OP.add)
            nc.vector.tensor_scalar_mul(K2sn[:], K2sn[:], 0.25)
            # split K2 into fp16 hi + lo
            K2hi = ldp.tile([S, KSZ], f16, tag="K2hi")
            nc.scalar.copy(out=K2hi[:], in_=K2sn[:])
            K2hi32 = ldp.tile([S, KSZ], f32, tag="K2hi32")
            nc.scalar.copy(out=K2hi32[:], in_=K2hi[:])
            K2lo = ldp.tile([S, KSZ], f16, tag="K2lo")
            nc.vector.tensor_tensor(
                out=K2lo[:], in0=K2sn[:], in1=K2hi32[:], op=OP.subtract)

            # ---------- x chain ----------
            X0 = ldp.tile([128, BPC * 32], f32, tag="X0")
            for b in range(BPC):
                nc.sync.dma_start(
                    out=X0[:, b * 32:(b + 1) * 32],
                    in_=_rap(x_d[b:b + 1, 0:1, 0:1],
                             [[8, 128], [1024, 4], [1, 8]]))
            PXp = pprep.tile([8, BPC * T], f32, tag="PXp")
            for b in range(BPC):
                for cc in range(4):
                    nc.tensor.transpose(
                        PXp[:, (b * 4 + cc) * 128:(b * 4 + cc + 1) * 128],
                        X0[:, (b * 4 + cc) * 8:(b * 4 + cc + 1) * 8],
                        ident[:, :])
            Xs8 = ldp.tile([8, BPC * T], f32, tag="Xs8")
            nc.scalar.copy(out=Xs8[:], in_=PXp[:])
            Xsig = ldp.tile([NSIG, T], f32, tag="Xsig")
            for b in range(BPC):
                nc.sync.dma_start(
                    out=Xsig[b * C:(b + 1) * C, :],
                    in_=Xs8[:, b * T:(b + 1) * T])

            xst = ldp.tile([NSIG, nc.vector.BN_STATS_DIM], f32, tag="xst")
            nc.vector.bn_stats(out=xst[:], in_=Xsig[:])
            mvx = ldp.tile([NSIG, nc.vector.BN_AGGR_DIM], f32, tag="mvx")
            nc.vector.bn_aggr(out=mvx[:], in_=xst[:])
            xstd = ldp.tile([NSIG, 1], f32, tag="xstd")
            nc.scalar.activation(out=xstd[:], in_=mvx[:, 1:2], func=ACT.Sqrt)
            nc.vector.tensor_scalar_add(xstd[:], xstd[:], EPS)
            xrstd = ldp.tile([NSIG, 1], f32, tag="xrstd")
            nc.vector.reciprocal(out=xrstd[:], in_=xstd[:])
            xbias = ldp.tile([NSIG, 1], f32, tag="xbias")
            nc.vector.scalar_tensor_tensor(
                out=xbias[:], in0=mvx[:, 0:1], scalar=-1.0, in1=xrstd[:],
                op0=OP.mult, op1=OP.mult)

            # Xnb: [xn-hat 0:512 |0| x2_hi 576:1088 |0| x2_lo 1152:1664 |0]
            nc.vector.memset(Xnb[:, 512:576], 0.0)
            nc.vector.memset(Xnb[:, 1088:1152], 0.0)
            nc.vector.memset(Xnb[:, 1664:1728], 0.0)
            nc.vector.tensor_scalar(
                out=Xnb[:, 0:T], in0=Xsig[:], scalar1=xrstd[:], scalar2=xbias[:],
                op0=OP.mult, op1=OP.add)
            x2f = ldp.tile([NSIG, T], f32, tag="x2f")
            nc.scalar.activation(out=x2f[:], in_=Xnb[:, 0:T], func=ACT.Square)
            nc.scalar.copy(out=Xnb[:, 576:1088], in_=x2f[:])
            x2hi32 = ldp.tile([NSIG, T], f32, tag="x2hi32")
            nc.scalar.copy(out=x2hi32[:], in_=Xnb[:, 576:1088])
            nc.vector.tensor_tensor(
                out=Xnb[:, 1152:1664], in0=x2f[:], in1=x2hi32[:],
                op=OP.subtract)
            # global fp16 ones buffer (H ones rows + F window-power taps)
            onesW = ldp.tile([NSIG, 1024], f16, tag="onesW")
            nc.vector.memset(onesW[:], 1.0)
            nc.scalar.dma_start(out=OnD[:], in_=onesW[:])

            # ---------- F build: quantized-tap transposes + staging ----------
            TPp = pprep.tile([128, 128], f16, tag="TPp")
            nc.tensor.transpose(TPp[:, 0:64], KNmh[:, 0:128], identh[:, :])
            nc.tensor.transpose(TPp[:, 64:128], KNmh[:, 128:256], identh[:, :])
            K2Tph = pprep.tile([KSZ, S], f16, tag="K2Tph")
            nc.tensor.transpose(K2Tph[:], K2hi[:], identh[:, :])
            K2Tpl = pprep.tile([KSZ, S], f16, tag="K2Tpl")
            nc.tensor.transpose(K2Tpl[:], K2lo[:], identh[:, :])
            TPsb = ldp.tile([128, 128], f16, tag="TPsb")
            nc.scalar.copy(out=TPsb[:], in_=TPp[:])
            K2sbh = ldp.tile([KSZ, S], f16, tag="K2sbh")
            nc.scalar.copy(out=K2sbh[:], in_=K2Tph[:])
            K2sbl = ldp.tile([KSZ, S], f16, tag="K2sbl")
            nc.scalar.copy(out=K2sbl[:], in_=K2Tpl[:])
            # FxD[kp*512 + c*64 + s] = KNmh[s, kp*8+c]
            nc.scalar.dma_start(
                out=_rap(FxD[0:1, 0:1], [[512, 16], [64, 8], [1, 64]]),
                in_=TPsb[:, 0:64])
            nc.scalar.dma_start(
                out=_rap(FxD[64:65, 0:1], [[512, 16], [64, 8], [1, 64]]),
                in_=TPsb[:, 64:128])
            # K2D rows: flat (kp, s) order == F col order (ch, j, s)
            nc.scalar.dma_start(out=K2D[0:1, :], in_=K2sbh[:])
            nc.scalar.dma_start(out=K2D[1:2, :], in_=K2sbl[:])

            # ---------- F_all scatter ----------
            nc.gpsimd.memset(F_all[:], 0.0)
            for j in range(4):
                # -2k taps
                nc.sync.dma_start(
                    out=_rap(F_all[8 * j:8 * j + 8, 64 * j:64 * j + 1],
                             [[2048, 8], [256, 8], [1, 64]]),
                    in_=_rap(FxD[4 * j:4 * j + 1, 0:1],
                             [[64, 8], [2048, 8], [1, 64]]))
                # window-power taps (hi rows then lo rows), all 1.0
                nc.scalar.dma_start(
                    out=_rap(F_all[32 + 8 * j:40 + 8 * j, 64 * j:64 * j + 1],
                             [[2048, 8], [256, 8], [1, 64]]),
                    in_=_rap(OnD[0:1, 0:1], [[512, 8], [64, 8], [1, 64]]))
                nc.scalar.dma_start(
                    out=_rap(F_all[64 + 8 * j:72 + 8 * j, 64 * j:64 * j + 1],
                             [[2048, 8], [256, 8], [1, 64]]),
                    in_=_rap(OnD[0:1, 0:1], [[512, 8], [64, 8], [1, 64]]))
            nc.sync.dma_start(out=F_all[96:98, :], in_=K2D[:])

            # ---------- stage signals + Hankel loads ----------
            for blk in range(3):
                nc.sync.dma_start(
                    out=_rap(XnD[0:1, blk * 576:blk * 576 + 1],
                             [[SIGW, NSIG], [1, 576]]),
                    in_=Xnb[:, blk * 576:(blk + 1) * 576])
            # ones rows of Hbig (rows 96-97), one DMA for all signals
            nc.scalar.dma_start(
                out=Hbig[96:98, :],
                in_=_rap(OnD[0:1, 0:1], [[8192, 2], [1, 8192]]))
            for sig in range(NSIG):
                eng = nc.sync if sig % 2 == 0 else nc.scalar
                eng.dma_start(
                    out=Hbig[0:96, sig * T:(sig + 1) * T],
                    in_=_rap(XnD[sig:sig + 1, 0:1],
                             [[576, 3], [1, 32], [1, 512]]))

        # ---------- main loop: matmuls + min + store ----------
        with tc.tile_pool(name="pmm", bufs=2, space="PSUM") as pmm:
            for b in range(BPC):
                for cc in range(4):
                    c0 = cc * 128
                    cnt = 128 if cc < 3 else TOUT - 3 * 128
                    acc = pmm.tile([128, 2048], f32, tag="acc")
                    for ch in range(C):
                        sig = b * C + ch
                        nc.tensor.matmul(
                            acc[:, ch * 256:(ch + 1) * 256],
                            lhsT=Hbig[0:98, sig * T + c0:sig * T + c0 + 128],
                            rhs=F_all[0:98, ch * 256:(ch + 1) * 256],
                            start=True, stop=True)
                    PM = outp.tile([128, KSZ], f32, tag="PM")
                    nc.vector.tensor_reduce(
                        out=PM[:],
                        in_=acc[:].rearrange("p (g s) -> p g s", s=S),
                        axis=AX.X, op=OP.min)
                    eng = nc.sync if (b * 4 + cc) % 2 == 0 else nc.scalar
                    eng.dma_start(out=o_d[b, c0:c0 + cnt, :], in_=PM[0:cnt, :])

    nc.compile()
    return nc


def get_nc():
    if "nc" not in _cache:
        _cache["nc"] = _build_nc()
    return _cache["nc"]


def kernel(x: np.ndarray, kernel: np.ndarray) -> np.ndarray:
    from concourse.bass_utils import run_bass_kernel_spmd

    nc = get_nc()
    x = np.ascontiguousarray(x, dtype=np.float32)
    kern = np.ascontiguousarray(kernel, dtype=np.float32)
    in_maps = [
        {"x": x[i * BPC:(i + 1) * BPC], "kernel": kern} for i in range(NCORES)
    ]
    res = run_bass_kernel_spmd(nc, in_maps, core_ids=list(range(NCORES)))
    return np.concatenate([r["out"] for r in res.results], axis=0)


if __name__ == "__main__":
    rng = np.random.default_rng(0)
    x = rng.standard_normal((B, T, C), dtype=np.float32)
    k = rng.uniform(-0.05, 0.05, (S, KSZ, C)).astype(np.float32)
    out = kernel(x=x, kernel=k)
    print(out.shape, out.dtype)


# revision 16
# speedup vs baseline: 2.9502x; 1.1445x over previous
"""Trainium2 Bass kernel for LocalSquaredDistanceLayer (shapelet min-distance).

Math (matches reference):
  out[b,t,k'] = min_s || xn[b, t+8j : t+8j+8, ch] - kern_n[s, k', :] ||^2
  with k' = 4*ch + j, xn z-normalized per (b,ch) over time, kern_n
  z-normalized per shapelet over (KSZ, C).

Decomposition: ||w - k||^2 = P + (||k||^2 - 2 w.k), where the window power
P[t,k'] = sum_c xn[t+8j+c]^2 is independent of the shapelet s, so it is
added AFTER the min.  The s-dependent part is one fp16 matmul per
(ch, t-chunk) against a 34-row operand.

Precision: everything derives from the SAME fp16-quantized x-hat / k-hat:
the matmul computes K2-hat - 2*x-hat.k-hat with exact fp16->fp32 products
(K2 carried as an fp16 hi+lo pair), and P-hat = sum x-hat^2 is computed in
fp32 on the vector engine.  The sum is exactly ||x-hat - k-hat||^2 up to
~2^-22 carriers, so the error is ~2*sqrt(D)*||quant|| ~ 1e-3 relative even
at the smallest minima (~0.11), safely under the 2e-2 gate.

Per-core layout (2 batches/core, kernel replicated; 8 cores data-parallel):
  Hbig [34, 16*512] fp16: rows 0-31 xn-hat shifts, rows 32-33 ones (K2 rows)
  F_all [34, 2048] fp16, col(ch,j,s) = ch*256 + j*64 + s:
    rows 8j+c: -2*k-hat taps;  rows 32/33: K2 hi/lo
  PqTs [128, 8*32] fp32: P-hat per (b,cc)-group, transposed in prep.
  Main loop per (b, t-chunk): 8 matmuls -> psum [128, 2048], DVE min-reduce
  over innermost 64 shapelets, DVE add of P, DMA out (gpsimd queue).
"""

import sys

for _p in ("/opt/trn_rl_repo",):
    if _p not in sys.path:
        sys.path.insert(0, _p)

import numpy as np

B, T, C = 16, 512, 8
S, KSZ = 64, 32
TOUT = T - KSZ + 1  # 481
NCORES = 8
BPC = B // NCORES  # batches per core
NSIG = BPC * C  # signals per core
EPS = 1e-8
SIGW = 544  # staged fp16 signal row: xn(512) + zero pad(32)

_cache = {}


def _rap(base, dims):
    """Raw AP at base slice's offset with explicit [step, count] dims (elems)."""
    import concourse.bass as bass

    return bass.AP(tensor=base.tensor, offset=base.offset, ap=[list(d) for d in dims])


def _build_nc():
    import concourse.bass as bass
    import concourse.bacc as bacc
    import concourse.tile as tile
    from concourse import mybir
    from concourse.masks import make_identity
    from contextlib import ExitStack

    f32 = mybir.dt.float32
    f16 = mybir.dt.float16
    AX = mybir.AxisListType
    OP = mybir.AluOpType
    ACT = mybir.ActivationFunctionType

    nc = bacc.Bacc("TRN2", target_bir_lowering=False, debug=False)
    x_d = nc.dram_tensor("x", [BPC, T, C], f32, kind="ExternalInput").ap()
    k_d = nc.dram_tensor("kernel", [S, KSZ, C], f32, kind="ExternalInput").ap()
    o_d = nc.dram_tensor("out", [BPC, TOUT, KSZ], f32, kind="ExternalOutput").ap()

    with tile.TileContext(nc) as tc, ExitStack() as ctx:
        const = ctx.enter_context(tc.tile_pool(name="const", bufs=1))
        outp = ctx.enter_context(tc.tile_pool(name="outp", bufs=3))
        dram = ctx.enter_context(tc.tile_pool(name="dram", bufs=1, space="DRAM"))

        ident = const.tile([128, 128], f32, tag="ident")
        make_identity(nc, ident[:])
        identh = const.tile([S, S], f16, tag="identh")
        make_identity(nc, identh[:])

        F_all = const.tile([34, 2048], f16, tag="F_all")
        Hbig = const.tile([34, NSIG * T], f16, tag="Hbig")
        Xnb = const.tile([NSIG, SIGW], f16, tag="Xnb")
        PqTs = const.tile([128, 8 * KSZ], f32, tag="PqTs")

        XnD = dram.tile([NSIG, SIGW], f16, tag="XnD")
        OnD = dram.tile([NSIG, 1024], f16, tag="OnD")
        FxD = dram.tile([128, 128], f16, tag="FxD")
        K2D = dram.tile([2, 2048], f16, tag="K2D")

        with tc.tile_pool(name="pprep", bufs=1, space="PSUM") as pprep, \
             tc.tile_pool(name="ldp", bufs=2) as ldp:
            # ---------- shapelet-kernel chain ----------
            KN = ldp.tile([S, KSZ * C], f32, tag="KN")
            nc.sync.dma_start(out=KN[:], in_=k_d.rearrange("s k c -> s (k c)"))
            kst = ldp.tile([S, nc.vector.BN_STATS_DIM], f32, tag="kst")
            nc.vector.bn_stats(out=kst[:], in_=KN[:])
            mvk = ldp.tile([S, nc.vector.BN_AGGR_DIM], f32, tag="mvk")
            nc.vector.bn_aggr(out=mvk[:], in_=kst[:])
            kstd = ldp.tile([S, 1], f32, tag="kstd")
            nc.scalar.activation(out=kstd[:], in_=mvk[:, 1:2], func=ACT.Sqrt)
            nc.vector.tensor_scalar_add(kstd[:], kstd[:], EPS)
            krstd = ldp.tile([S, 1], f32, tag="krstd")
            nc.vector.reciprocal(out=krstd[:], in_=kstd[:])
            # KNmh = fp16(-2 * (KN - mean) * rstd)   (the k-hat taps)
            kscale = ldp.tile([S, 1], f32, tag="kscale")
            nc.vector.tensor_scalar_mul(kscale[:], krstd[:], -2.0)
            kbias = ldp.tile([S, 1], f32, tag="kbias")
            nc.vector.scalar_tensor_tensor(
                out=kbias[:], in0=mvk[:, 0:1], scalar=2.0, in1=krstd[:],
                op0=OP.mult, op1=OP.mult)
            KNmh = ldp.tile([S, KSZ * C], f16, tag="KNmh")
            nc.vector.tensor_scalar(
                out=KNmh[:], in0=KN[:], scalar1=kscale[:], scalar2=kbias[:],
                op0=OP.mult, op1=OP.add)
            # K2[s,k'] = 0.25 * sum_c KNmh^2 in fp32 (exact squares of taps)
            KN2 = ldp.tile([S, KSZ * C], f32, tag="KN2")
            nc.scalar.activation(out=KN2[:], in_=KNmh[:], func=ACT.Square)
            K2sn = ldp.tile([S, KSZ], f32, tag="K2sn")
            nc.vector.tensor_reduce(
                out=K2sn[:], in_=KN2[:].rearrange("s (k c) -> s k c", c=C),
                axis=AX.X, op=OP.add)
            nc.vector.tensor_scalar_mul(K2sn[:], K2sn[:], 0.25)
            # split K2 into fp16 hi + lo
            K2hi = ldp.tile([S, KSZ], f16, tag="K2hi")
            nc.scalar.copy(out=K2hi[:], in_=K2sn[:])
            K2hi32 = ldp.tile([S, KSZ], f32, tag="K2hi32")
            nc.scalar.copy(out=K2hi32[:], in_=K2hi[:])
            K2lo = ldp.tile([S, KSZ], f16, tag="K2lo")
            nc.vector.tensor_tensor(
                out=K2lo[:], in0=K2sn[:], in1=K2hi32[:], op=OP.subtract)

            # ---------- F build: quantized-tap transposes + staging ----------
            TPp = pprep.tile([128, 128], f16, tag="TPp")
            nc.tensor.transpose(TPp[:, 0:64], KNmh[:, 0:128], identh[:, :])
            nc.tensor.transpose(TPp[:, 64:128], KNmh[:, 128:256], identh[:, :])
            K2Tph = pprep.tile([KSZ, S], f16, tag="K2Tph")
            nc.tensor.transpose(K2Tph[:], K2hi[:], identh[:, :])
            K2Tpl = pprep.tile([KSZ, S], f16, tag="K2Tpl")
            nc.tensor.transpose(K2Tpl[:], K2lo[:], identh[:, :])
            TPsb = ldp.tile([128, 128], f16, tag="TPsb")
            nc.scalar.copy(out=TPsb[:], in_=TPp[:])
            K2sb = ldp.tile([KSZ, 128], f16, tag="K2sb")
            nc.scalar.copy(out=K2sb[:, 0:64], in_=K2Tph[:])
            nc.scalar.copy(out=K2sb[:, 64:128], in_=K2Tpl[:])
            # FxD[kp*512 + c*64 + s] = KNmh[s, kp*8+c]
            nc.scalar.dma_start(
                out=_rap(FxD[0:1, 0:1], [[512, 16], [64, 8], [1, 64]]),
                in_=TPsb[:, 0:64])
            nc.scalar.dma_start(
                out=_rap(FxD[64:65, 0:1], [[512, 16], [64, 8], [1, 64]]),
                in_=TPsb[:, 64:128])
            # K2D[h, kp*64+s] = K2sb[kp, h*64+s]  (hi row then lo row),
            # iterated partition(kp)-major on the SBUF side
            nc.scalar.dma_start(
                out=_rap(K2D[0:1, 0:1], [[64, 32], [2048, 2], [1, 64]]),
                in_=_rap(K2sb[0:1, 0:1], [[128, 32], [64, 2], [1, 64]]))
            # F_all scatter: per-j taps blocks; other-j blocks must be zero
            nc.gpsimd.memset(F_all[:], 0.0)
            for j in range(4):
                nc.sync.dma_start(
                    out=_rap(F_all[8 * j:8 * j + 8, 64 * j:64 * j + 1],
                             [[2048, 8], [256, 8], [1, 64]]),
                    in_=_rap(FxD[4 * j:4 * j + 1, 0:1],
                             [[64, 8], [2048, 8], [1, 64]]))
            nc.scalar.dma_start(out=F_all[32:34, :], in_=K2D[:])

            # ---------- x chain ----------
            X0 = ldp.tile([128, BPC * 32], f32, tag="X0")
            for b in range(BPC):
                nc.sync.dma_start(
                    out=X0[:, b * 32:(b + 1) * 32],
                    in_=_rap(x_d[b:b + 1, 0:1, 0:1],
                             [[8, 128], [1024, 4], [1, 8]]))
            PXp = pprep.tile([8, BPC * T], f32, tag="PXp")
            for b in range(BPC):
                for cc in range(4):
                    nc.tensor.transpose(
                        PXp[:, (b * 4 + cc) * 128:(b * 4 + cc + 1) * 128],
                        X0[:, (b * 4 + cc) * 8:(b * 4 + cc + 1) * 8],
                        ident[:, :])
            Xs8 = ldp.tile([8, BPC * T], f32, tag="Xs8")
            nc.scalar.copy(out=Xs8[:], in_=PXp[:])
            Xsig = ldp.tile([NSIG, T], f32, tag="Xsig")
            for b in range(BPC):
                nc.scalar.dma_start(
                    out=Xsig[b * C:(b + 1) * C, :],
                    in_=Xs8[:, b * T:(b + 1) * T])

            xst = ldp.tile([NSIG, nc.vector.BN_STATS_DIM], f32, tag="xst")
            nc.vector.bn_stats(out=xst[:], in_=Xsig[:])
            mvx = ldp.tile([NSIG, nc.vector.BN_AGGR_DIM], f32, tag="mvx")
            nc.vector.bn_aggr(out=mvx[:], in_=xst[:])
            xstd = ldp.tile([NSIG, 1], f32, tag="xstd")
            nc.scalar.activation(out=xstd[:], in_=mvx[:, 1:2], func=ACT.Sqrt)
            nc.vector.tensor_scalar_add(xstd[:], xstd[:], EPS)
            xrstd = ldp.tile([NSIG, 1], f32, tag="xrstd")
            nc.vector.reciprocal(out=xrstd[:], in_=xstd[:])
            xbias = ldp.tile([NSIG, 1], f32, tag="xbias")
            nc.vector.scalar_tensor_tensor(
                out=xbias[:], in0=mvx[:, 0:1], scalar=-1.0, in1=xrstd[:],
                op0=OP.mult, op1=OP.mult)

            # x-hat (fp16) + zero pad; stage to DRAM for the Hankel loads
            nc.vector.memset(Xnb[:, 512:SIGW], 0.0)
            nc.vector.tensor_scalar(
                out=Xnb[:, 0:T], in0=Xsig[:], scalar1=xrstd[:], scalar2=xbias[:],
                op0=OP.mult, op1=OP.add)
            nc.sync.dma_start(
                out=_rap(XnD[0:1, 0:1], [[SIGW, NSIG], [1, SIGW]]),
                in_=Xnb[:, :])
            onesW = ldp.tile([NSIG, 1024], f16, tag="onesW")
            nc.gpsimd.memset(onesW[:], 1.0)
            nc.scalar.dma_start(out=OnD[:], in_=onesW[:])

            # P-hat path: x2f = x-hat^2 (fp32, exact), sliding window-8 sums
            x2f = ldp.tile([NSIG, SIGW], f32, tag="x2f")
            nc.scalar.activation(out=x2f[:], in_=Xnb[:, :], func=ACT.Square)
            Pw1 = ldp.tile([NSIG, SIGW], f32, tag="Pw1")
            nc.vector.tensor_tensor(
                out=Pw1[:, 0:543], in0=x2f[:, 0:543], in1=x2f[:, 1:544],
                op=OP.add)
            Pw2 = ldp.tile([NSIG, SIGW], f32, tag="Pw2")
            nc.vector.tensor_tensor(
                out=Pw2[:, 0:541], in0=Pw1[:, 0:541], in1=Pw1[:, 2:543],
                op=OP.add)
            Pw = ldp.tile([NSIG, SIGW], f32, tag="Pw")
            nc.vector.tensor_tensor(
                out=Pw[:, 0:537], in0=Pw2[:, 0:537], in1=Pw2[:, 4:541],
                op=OP.add)
            # Pq[b][ch*4+j, u] = Pw[b*8+ch, u+8j]  (SBUF->SBUF partition remap)
            Pq = [ldp.tile([KSZ, SIGW], f32, tag=f"Pq{b}", name=f"Pq{b}")
                  for b in range(BPC)]
            for b in range(BPC):
                nc.sync.dma_start(
                    out=Pq[b][:, 0:512],
                    in_=_rap(Pw[b * C:b * C + 1, 0:1],
                             [[SIGW, 8], [8, 4], [1, 512]]))
            # transpose P per (b,cc) group into [t, g] orientation
            PqTp = pprep.tile([128, 8 * KSZ], f32, tag="PqTp")
            for b in range(BPC):
                for cc in range(4):
                    grp = b * 4 + cc
                    nc.tensor.transpose(
                        PqTp[:, grp * KSZ:(grp + 1) * KSZ],
                        Pq[b][:, cc * 128:(cc + 1) * 128],
                        ident[0:KSZ, 0:KSZ])
            nc.scalar.copy(out=PqTs[:], in_=PqTp[:])

            # ---------- Hankel loads (2 signals per DMA) + ones rows ----------
            nc.scalar.dma_start(
                out=Hbig[32:34, :],
                in_=_rap(OnD[0:1, 0:1], [[8192, 2], [1, 8192]]))
            for i in range(8):
                eng = nc.sync if i % 2 == 0 else nc.scalar
                eng.dma_start(
                    out=_rap(Hbig[0:1, 2 * i * T:2 * i * T + 1],
                             [[NSIG * T, KSZ], [T, 2], [1, T]]),
                    in_=_rap(XnD[2 * i:2 * i + 1, 0:1],
                             [[1, KSZ], [SIGW, 2], [1, T]]))

        # ---------- main loop: matmuls + min + P add + store ----------
        with tc.tile_pool(name="pmm", bufs=2, space="PSUM") as pmm:
            for b in range(BPC):
                for cc in range(4):
                    grp = b * 4 + cc
                    c0 = cc * 128
                    cnt = 128 if cc < 3 else TOUT - 3 * 128
                    acc = pmm.tile([128, 2048], f32, tag="acc")
                    for ch in range(C):
                        sig = b * C + ch
                        nc.tensor.matmul(
                            acc[:, ch * 256:(ch + 1) * 256],
                            lhsT=Hbig[0:34, sig * T + c0:sig * T + c0 + 128],
                            rhs=F_all[0:34, ch * 256:(ch + 1) * 256],
                            start=True, stop=True)
                    PM = outp.tile([128, KSZ], f32, tag="PM")
                    nc.vector.tensor_reduce(
                        out=PM[:],
                        in_=acc[:].rearrange("p (g s) -> p g s", s=S),
                        axis=AX.X, op=OP.min)
                    PMf = outp.tile([128, KSZ], f32, tag="PMf")
                    nc.vector.tensor_tensor(
                        out=PMf[:], in0=PM[:],
                        in1=PqTs[:, grp * KSZ:(grp + 1) * KSZ], op=OP.add)
                    nc.gpsimd.dma_start(out=o_d[b, c0:c0 + cnt, :],
                                        in_=PMf[0:cnt, :])

    nc.compile()
    return nc


def get_nc():
    if "nc" not in _cache:
        _cache["nc"] = _build_nc()
    return _cache["nc"]


def kernel(x: np.ndarray, kernel: np.ndarray) -> np.ndarray:
    from concourse.bass_utils import run_bass_kernel_spmd

    nc = get_nc()
    x = np.ascontiguousarray(x, dtype=np.float32)
    kern = np.ascontiguousarray(kernel, dtype=np.float32)
    in_maps = [
        {"x": x[i * BPC:(i + 1) * BPC], "kernel": kern} for i in range(NCORES)
    ]
    res = run_bass_kernel_spmd(nc, in_maps, core_ids=list(range(NCORES)))
    return np.concatenate([r["out"] for r in res.results], axis=0)


if __name__ == "__main__":
    rng = np.random.default_rng(0)
    x = rng.standard_normal((B, T, C), dtype=np.float32)
    k = rng.uniform(-0.05, 0.05, (S, KSZ, C)).astype(np.float32)
    out = kernel(x=x, kernel=k)
    print(out.shape, out.dtype)
